# revision 42
# baseline (speedup 1.0000x reference)
"""Trainium2 Bass kernel for the BN + 1x1-conv self-attention block.

Reference computation (per batch item, c=256 channels, n=4096 tokens):
  BN(x) over (b,h,w) -> qkv = W_qkv @ xn -> attention -> W_out proj -> +x

Sharding: 8 cores = 4 batch items x 2 query-halves. Each core:
  - holds the full x of its batch item in [c, pos] layout, rotated so its
    2048 query positions come first (attention is permutation-invariant
    in the key axis, so the rotation only needs consistency of K and V)
  - computes partial BN sums over its 2048 positions; a [128,4] AllGather
    across all 8 cores yields exact global batch statistics
  - folds BN into the QKV conv: W' = W * s_c (per input channel),
    b' = b_qkv + W @ t. The V-channel shift contributes a constant
    per-channel offset to the attention output (softmax weights sum to 1);
    it commutes through W_out, so it is pre-folded into the output bias:
    b_out' = b_out + W_out @ b_v.
  - attention is computed in transposed form: S^T[k,q] tiles flow
    scores -> exp -> (A^T V -> out^T[c,q]). Score k-tiles are PAIRED into
    [128,1024] PSUM tiles so one ACT exp covers two k-tiles. The softmax
    denominator accumulates on DVE in bf16 (2x perf mode) over the wide
    exp pairs; the partition reduction is two accumulating ones-matmuls.
    The softmax normalization (x 1/denominator, per query) commutes past
    the output projection, so attention PSUM evacuates with plain copies
    and the scale is applied once on the projected output.
    The pair loop is software-pipelined: AV(p-1) runs on PE while ACT
    computes exp(p), so PE has no exp-latency bubble.

All heavy matmuls use bfloat16 operands. Logits need no max-subtraction:
they are ~N(0,1) after BN, so exp() is safe in fp32.
"""
import sys

sys.path.append("/opt/trn_rl_repo")

import numpy as np
from contextlib import ExitStack

import concourse.bass as bass
import concourse.tile as tile
from concourse import bacc, mybir
from concourse import bass_utils

F32 = mybir.dt.float32
F32R = mybir.dt.float32r
F8 = mybir.dt.float8e4
DR = mybir.MatmulPerfMode.DoubleRow
AF = mybir.ActivationFunctionType
ALU = mybir.AluOpType

B, C, H, W = 4, 256, 64, 64
NPOS = H * W          # 4096 positions per item
NQ = NPOS // 2        # 2048 query positions per core
N_CORES = 8
CT = C // 128         # 2 channel partition-tiles
OT = 3 * C // 128     # 6 qkv output tiles
EPS = 1e-5
SCALE = C ** (-0.5)   # 1/16
NTOT = float(B * NPOS)  # BN normalizer 16384
N_WARMUP_MM = 120
MM_DT = mybir.dt.bfloat16    # dtype for x / conv-weight matmul operands
NPAIR = 16            # 32 k-tiles as 16 pairs per query chunk
EXP_SHIFT = -3.4657359  # -ln(32): keeps exp() under the fp8e4 max (448);
                        # a uniform scale on exp cancels in the softmax


def _build(n_reps: int = 1, n_qc: int = 4):
    nc = bacc.Bacc("TRN2", target_bir_lowering=False, debug=False)

    x_full = nc.dram_tensor("x_full", [C, NPOS], F32, kind="ExternalInput")
    w_qkv_t = nc.dram_tensor("w_qkv_t", [C, 3 * C], F32, kind="ExternalInput")
    w_out_t = nc.dram_tensor("w_out_t", [C, C], F32, kind="ExternalInput")
    b_qkv = nc.dram_tensor("b_qkv", [3 * C, 1], F32, kind="ExternalInput")
    b_out = nc.dram_tensor("b_out", [C, 1], F32, kind="ExternalInput")
    gamma = nc.dram_tensor("gamma", [C, 1], F32, kind="ExternalInput")
    beta = nc.dram_tensor("beta", [C, 1], F32, kind="ExternalInput")
    out_d = nc.dram_tensor("out", [C, NQ], mybir.dt.bfloat16, kind="ExternalOutput")

    with tile.TileContext(nc) as tc:
        with ExitStack() as ctx:
            big = ctx.enter_context(tc.tile_pool(name="big", bufs=1))
            xqp = ctx.enter_context(tc.tile_pool(name="xqp", bufs=2))
            stage = ctx.enter_context(tc.tile_pool(name="stage", bufs=2))
            vec = ctx.enter_context(tc.tile_pool(name="vec", bufs=1))
            expp = ctx.enter_context(tc.tile_pool(name="expp", bufs=17))
            attnp = ctx.enter_context(tc.tile_pool(name="attnp", bufs=2))
            outp = ctx.enter_context(tc.tile_pool(name="outp", bufs=4))
            dram = ctx.enter_context(tc.tile_pool(name="dram", bufs=1, space="DRAM"))
            ps_s = ctx.enter_context(tc.tile_pool(name="ps_s", bufs=2, space="PSUM"))
            ps_av = ctx.enter_context(tc.tile_pool(name="ps_av", bufs=2, space="PSUM"))
            ps_x = ctx.enter_context(tc.tile_pool(name="ps_x", bufs=2, space="PSUM"))

            # DMA triggers only on sync/scalar: the collective's trigger+wait
            # occupies the gpsimd FIFO, so anything queued behind it stalls.
            dma_engs = [nc.sync, nc.scalar]

            for _rep in range(n_reps):
                # ---- x query-half first: stats feed the AllReduce ASAP ----
                xq = []
                for ct in range(CT):
                    xqt = xqp.tile([128, NQ], F32, tag=f"xq_{ct}", name=f"xq_{ct}")
                    # chunked so bn_stats can start on early chunks; spread
                    # across engine DMA queues
                    for i in range(4):
                        eng = dma_engs[(4 * ct + i) % 2]
                        eng.dma_start(
                            xqt[:, 512 * i:512 * (i + 1)],
                            x_full[128 * ct:128 * (ct + 1), 512 * i:512 * (i + 1)])
                    xq.append(xqt)

                statp = vec.tile([128, 4], F32, tag="statp")
                for ct in range(CT):
                    xg = xq[ct][:].rearrange("p (n f) -> p n f", f=512)
                    stats = vec.tile([128, 4, 6], F32, tag="bnstats")
                    for i in range(4):
                        nc.vector.bn_stats(out=stats[:, i, :], in_=xg[:, i, :])
                    mv = vec.tile([128, 2], F32, tag="bnmv")
                    nc.vector.bn_aggr(out=mv[:], in_=stats[:])
                    # shard sums: sum = mean*2048 ; sumsq = (var + mean^2)*2048
                    nc.scalar.mul(statp[:, 2 * ct:2 * ct + 1], mv[:, 0:1], float(NQ))
                    m2 = vec.tile([128, 1], F32, tag="m2")
                    nc.vector.tensor_mul(m2[:], mv[:, 0:1], mv[:, 0:1])
                    nc.vector.tensor_add(m2[:], m2[:], mv[:, 1:2])
                    nc.scalar.mul(statp[:, 2 * ct + 1:2 * ct + 2], m2[:], float(NQ))

                # AllGather + local sum: ~2x lower latency than AllReduce
                cc_in = dram.tile([128, 4], F32, name="cc_in")
                cc_out = dram.tile([N_CORES * 128, 4], F32, name="cc_out")
                nc.sync.dma_start(cc_in[:], statp[:])
                nc.gpsimd.collective_compute(
                    "AllGather",
                    ALU.bypass,
                    replica_groups=[list(range(N_CORES))],
                    ins=[cc_in.opt()],
                    outs=[cc_out.opt()],
                )

                # PE warm-up while DMAs + the collective are in flight
                # (memset-sourced operands: independent of input DMAs).
                # Only the first body needs it: in steady state the previous
                # body's attention keeps the PE HAM-warm.
                if _rep == 0:
                    warm_f = stage.tile([128, 512], F32, tag="warm_f", name="warm_f", bufs=1)
                    warm_r = stage.tile([128, 512], MM_DT, tag="warm_r", name="warm_r", bufs=1)
                    nc.vector.memset(warm_f[:], 0.5)
                    nc.vector.tensor_copy(warm_r[:], warm_f[:])
                    for wi in range(N_WARMUP_MM):
                        pool = ps_s if wi % 2 == 0 else ps_x
                        pw = pool.tile([128, 512], F32, tag="s" if wi % 2 == 0 else "x",
                                       name=f"warm_{wi}")
                        nc.tensor.matmul(
                            pw[:], warm_r[:, 0:128], warm_r[:],
                            start=True, stop=True,
                        )

                # ---- overlapped with the AllReduce: weights, x rounding ----
                w_f32 = []
                for ct in range(CT):
                    wt = big.tile([128, 3 * C], F32, tag=f"w_f32_{ct}", name=f"w_f32_{ct}")
                    nc.sync.dma_start(wt[:], w_qkv_t[128 * ct:128 * (ct + 1), :])
                    w_f32.append(wt)
                wout_r = []
                for ct in range(CT):
                    ws = stage.tile([128, C], F32, tag="wout_stage", name="wout_stage")
                    nc.scalar.dma_start(ws[:], w_out_t[128 * ct:128 * (ct + 1), :])
                    wr = big.tile([128, C], MM_DT, tag=f"wout_r_{ct}", name=f"wout_r_{ct}")
                    nc.vector.tensor_copy(wr[:], ws[:])
                    wout_r.append(wr)

                bq_col, bo_col = [], []
                for ot in range(OT):
                    t = vec.tile([128, 1], F32, tag=f"bq_col_{ot}", name=f"bq_col_{ot}")
                    nc.scalar.dma_start(t[:], b_qkv[128 * ot:128 * (ot + 1), :])
                    bq_col.append(t)
                for ct in range(CT):
                    t = vec.tile([128, 1], F32, tag=f"bo_{ct}", name=f"bo_{ct}")
                    nc.sync.dma_start(t[:], b_out[128 * ct:128 * (ct + 1), :])
                    bo_col.append(t)
                # gamma/beta as [128, 2] (channel-tile on the free dim)
                ga2 = vec.tile([128, CT], F32, tag="ga2")
                nc.scalar.dma_start(ga2[:], gamma[:].rearrange("(c p) one -> p (c one)", p=128))
                be2 = vec.tile([128, CT], F32, tag="be2")
                nc.sync.dma_start(be2[:], beta[:].rearrange("(c p) one -> p (c one)", p=128))

                eps_col = vec.tile([128, 1], F32, tag="eps_col")
                nc.vector.memset(eps_col[:], EPS)
                # dummy Exp: pulls the one ACT table load off the exp stream.
                # The stats chain uses no other ACT function, so the table
                # set never switches after this.
                exp_warm = vec.tile([128, 1], F32, tag="exp_warm")
                nc.scalar.activation(exp_warm[:], eps_col[:], AF.Exp)
                magic = vec.tile([128, CT], mybir.dt.int32, tag="magic")
                nc.vector.memset(magic[:], 0x5F3759DF)
                # fp8 ones for the DoubleRow denominator matmul; dim1 step
                # must be 16-byte aligned, hence the [128, 2, 16] layout
                ones8 = vec.tile([128, 2, 16], F8, tag="ones8")
                nc.vector.memset(ones8[:], 1.0)
                shift_col = vec.tile([128, 1], F32, tag="shift_col")
                nc.vector.memset(shift_col[:], EXP_SHIFT)
                ones_row_f = vec.tile([1, 128], F32, tag="ones_row_f")
                nc.vector.memset(ones_row_f[:], 1.0)
                ones_row_r = vec.tile([1, 128], F32R, tag="ones_row_r")
                nc.vector.tensor_copy(ones_row_r[:], ones_row_f[:])

                # x in bf16 (QKV inputs stay bf16 for accuracy margin; fp8 is
                # reserved for the attention inner loops). Rounding copies on
                # DVE; gpsimd stays free so the collective doorbell fires
                # immediately.
                x_r = []
                for ct in range(CT):
                    xrt = big.tile([128, NPOS], MM_DT, tag=f"x_r_{ct}", name=f"x_r_{ct}")
                    nc.vector.tensor_copy(xrt[:, 0:NQ], xq[ct][:])
                    for h2 in range(2):
                        st = stage.tile([128, NQ // 2], F32, tag="x_stage", name="x_stage")
                        # second chunk rides the gpsimd DMA queue: it is only
                        # needed after the collective wait clears, so the
                        # blocked gpsimd FIFO costs nothing and the sync/
                        # scalar queues shed a third of their load
                        eng = dma_engs[ct % 2] if h2 == 0 else nc.gpsimd
                        lo = NQ + (NQ // 2) * h2
                        eng.dma_start(st[:], x_full[128 * ct:128 * (ct + 1), lo:lo + NQ // 2])
                        nc.vector.tensor_copy(xrt[:, lo:lo + NQ // 2], st[:])
                    x_r.append(xrt)

                g_all = vec.tile([128, N_CORES, 4], F32, tag="g_all")
                nc.sync.dma_start(
                    g_all[:],
                    cc_out[:].rearrange("(r p) c -> p r c", p=128),
                )
                # local tree-sum over the 8 gathered shards
                nc.vector.tensor_add(g_all[:, 0:4, :], g_all[:, 0:4, :], g_all[:, 4:8, :])
                nc.vector.tensor_add(g_all[:, 0:2, :], g_all[:, 0:2, :], g_all[:, 2:4, :])
                g_stats = vec.tile([128, CT, 2], F32, tag="g_stats")
                nc.vector.tensor_tensor(
                    out=g_stats[:],
                    in0=g_all[:, 0, :].rearrange("p (c two) -> p c two", two=2),
                    in1=g_all[:, 1, :].rearrange("p (c two) -> p c two", two=2),
                    op=ALU.add,
                )

                # ---- derive s (scale) and t (shift), both tiles at once ----
                mean2 = vec.tile([128, CT], F32, tag="mean2")
                nc.vector.tensor_single_scalar(
                    out=mean2[:], in_=g_stats[:, :, 0], scalar=1.0 / NTOT, op=ALU.mult)
                e2t = vec.tile([128, CT], F32, tag="e2t")
                nc.vector.tensor_single_scalar(
                    out=e2t[:], in_=g_stats[:, :, 1], scalar=1.0 / NTOT, op=ALU.mult)
                var2 = vec.tile([128, CT], F32, tag="var2")
                nc.vector.tensor_mul(var2[:], mean2[:], mean2[:])
                nc.vector.tensor_tensor(out=var2[:], in0=e2t[:], in1=var2[:], op=ALU.subtract)
                nc.vector.tensor_single_scalar(
                    out=var2[:], in_=var2[:], scalar=EPS, op=ALU.add)
                # 1/std via fast-inverse-sqrt on DVE (magic seed + 3 Newton
                # steps, ~4e-6 rel err): no Sqrt table load on ACT, so the
                # Exp table set never switches between bodies.
                s2 = vec.tile([128, CT], F32, tag="s2")
                s2i = s2[:].bitcast(mybir.dt.int32)
                nc.vector.tensor_single_scalar(
                    out=s2i, in_=var2[:].bitcast(mybir.dt.int32), scalar=1,
                    op=ALU.logical_shift_right)
                nc.vector.tensor_tensor(out=s2i, in0=magic[:], in1=s2i, op=ALU.subtract)
                nwt = vec.tile([128, CT], F32, tag="nwt")
                for _ in range(3):
                    nc.vector.tensor_mul(nwt[:], s2[:], s2[:])
                    nc.vector.tensor_mul(nwt[:], nwt[:], var2[:])
                    nc.vector.tensor_scalar(
                        out=nwt[:], in0=nwt[:], scalar1=-0.5, scalar2=1.5,
                        op0=ALU.mult, op1=ALU.add)
                    nc.vector.tensor_mul(s2[:], s2[:], nwt[:])
                nc.vector.tensor_mul(s2[:], s2[:], ga2[:])
                t2 = vec.tile([128, CT], F32, tag="t2")
                nc.vector.tensor_mul(t2[:], mean2[:], s2[:])
                nc.vector.tensor_tensor(out=t2[:], in0=be2[:], in1=t2[:], op=ALU.subtract)
                s_col = [s2[:, ct:ct + 1] for ct in range(CT)]
                t_col = [t2[:, ct:ct + 1] for ct in range(CT)]

                # ---- fold BN into weights ----
                bq_fold = []
                for ot in range(OT):
                    pbq = ps_x.tile([128, 1], F32, tag="x", name=f"pbq_{ot}")
                    for ct in range(CT):
                        nc.tensor.matmul(
                            pbq[:],
                            w_f32[ct][:, 128 * ot:128 * (ot + 1)],
                            t_col[ct],
                            start=(ct == 0), stop=(ct == CT - 1),
                        )
                    bqf = vec.tile([128, 1], F32, tag=f"bqf_{ot}", name=f"bqf_{ot}")
                    nc.vector.tensor_add(bqf[:], pbq[:], bq_col[ot][:])
                    bq_fold.append(bqf)
                wqkv_r = []
                for ct in range(CT):
                    wr = big.tile([128, 3 * C], MM_DT, tag=f"wqkv_r_{ct}", name=f"wqkv_r_{ct}")
                    nc.vector.tensor_scalar_mul(wr[:], w_f32[ct][:], s_col[ct])
                    wqkv_r.append(wr)

                # b_out' = b_out + W_out @ b_v  (the V-channel BN shift applied
                # post-attention commutes through the output projection)
                bv_r = []
                for ct in range(CT):
                    t = vec.tile([128, 1], MM_DT, tag=f"bv_r_{ct}", name=f"bv_r_{ct}")
                    nc.vector.tensor_copy(t[:], bq_fold[4 + ct][:])
                    bv_r.append(t)
                bo2 = []
                for ot in range(CT):
                    pbo = ps_x.tile([128, 1], F32, tag="x", name=f"pbo_{ot}")
                    for ct in range(CT):
                        nc.tensor.matmul(
                            pbo[:],
                            wout_r[ct][:, 128 * ot:128 * (ot + 1)],
                            bv_r[ct][:],
                            start=(ct == 0), stop=(ct == CT - 1),
                        )
                    t = vec.tile([128, 1], F32, tag=f"bo2_{ot}", name=f"bo2_{ot}")
                    nc.vector.tensor_add(t[:], pbo[:], bo_col[ot][:])
                    bo2.append(t)

                # ---- QKV projections ----
                # Q/K evacuate PSUM via ACT (fused per-partition bias add)
                # into fp8, laid out [128, ct, pos] so DoubleRow matmuls can
                # take the 256-channel contraction in one instruction.
                # V evacuates via DVE plain copy into fp8 [128, kt, ch].
                qt8 = big.tile([128, CT, NQ], F8, tag="qt8", name="qt8")
                k8 = big.tile([128, CT, NPOS], F8, tag="k8", name="k8")
                qkv_i = 0
                for ot in range(4):  # o-tiles 0,1 -> Q ; 2,3 -> K
                    is_q = ot < 2
                    npc = 4 if is_q else 8
                    for pc in range(npc):
                        pool = ps_s if qkv_i % 2 == 0 else ps_x
                        tg = "s" if qkv_i % 2 == 0 else "x"
                        qkv_i += 1
                        ps = pool.tile([128, 512], F32, tag=tg, name=f"qkv_{ot}_{pc}")
                        for ct in range(CT):
                            nc.tensor.matmul(
                                ps[:],
                                wqkv_r[ct][:, 128 * ot:128 * (ot + 1)],
                                x_r[ct][:, 512 * pc:512 * (pc + 1)],
                                start=(ct == 0), stop=(ct == CT - 1),
                            )
                        # Q evacuates on DVE, K on ACT: splits the PSUM->fp8
                        # bias-add load across both engines
                        if is_q:
                            dest = qt8[:, ot, 512 * pc:512 * (pc + 1)]
                            nc.vector.tensor_scalar_add(dest, ps[:], bq_fold[ot][:])
                        else:
                            dest = k8[:, ot - 2, 512 * pc:512 * (pc + 1)]
                            nc.scalar.activation(dest, ps[:], AF.Identity,
                                                 bias=bq_fold[ot][:])
                v8 = big.tile([128, 32, C], F8, tag="v8")
                for pt in range(32):
                    psv = ps_x.tile([128, C], F32, tag="x", name=f"v_{pt}")
                    for ct in range(CT):
                        nc.tensor.matmul(
                            psv[:],
                            x_r[ct][:, 128 * pt:128 * (pt + 1)],
                            wqkv_r[ct][:, 2 * C:3 * C],
                            start=(ct == 0), stop=(ct == CT - 1),
                        )
                    nc.vector.tensor_copy(v8[:, pt, :], psv[:])

                # ---- attention, software-pipelined over 16 k-tile pairs ----
                # DoubleRow fp8 matmuls take the full 256-deep contraction
                # (channels for scores, a k-tile pair for AV/denominator) in
                # one instruction each.
                def scores_pair(qs, p, qc):
                    ss = ps_s.tile([128, 2, 512], F32, tag="s", name=f"ss_{qc}_{p}")
                    for half in range(2):
                        kt = 2 * p + half
                        nc.tensor.matmul(
                            ss[:, half, :],
                            k8[:, :, 128 * kt:128 * (kt + 1)],
                            qt8[:, :, qs],
                            start=True, stop=True, perf_mode=DR,
                        )
                    ex = expp.tile([128, 2, 512], F8, tag="ex", name=f"ex_{qc}_{p}")
                    nc.scalar.activation(ex[:], ss[:], AF.Exp, scale=SCALE,
                                         bias=shift_col[:])
                    return ex

                for qc in range(n_qc):
                    qs = slice(512 * qc, 512 * (qc + 1))
                    av = [ps_av.tile([128, 512], F32, tag="av", name=f"av_{qc}_{i}")
                          for i in range(CT)]
                    # softmax denominator: DoubleRow ones-matmuls, batched
                    # back-to-back after the pair loop so the ones stationary
                    # operand loads once (consecutive identical lhsT skips
                    # LDWEIGHTS) instead of re-loading between av/score MMs
                    dnps = ps_x.tile([1, 512], F32, tag="x", name=f"dnps_{qc}")

                    def av_step(ex, p, av=av):
                        for ct in range(CT):
                            nc.tensor.matmul(
                                av[ct][:],
                                v8[:, 2 * p:2 * p + 2, 128 * ct:128 * (ct + 1)],
                                ex[:],
                                start=(p == 0), stop=(p == NPAIR - 1),
                                perf_mode=DR,
                            )

                    exs = [scores_pair(qs, 0, qc)]
                    for p in range(1, NPAIR):
                        exs.append(scores_pair(qs, p, qc))
                        av_step(exs[p - 1], p - 1)
                    av_step(exs[NPAIR - 1], NPAIR - 1)
                    for p in range(NPAIR):
                        nc.tensor.matmul(
                            dnps[:], ones8[:, :, 0:1], exs[p][:],
                            start=(p == 0), stop=(p == NPAIR - 1),
                            perf_mode=DR,
                        )

                    # 1/denominator, broadcast to all partitions via ones-matmul
                    rec = vec.tile([1, 512], F32, tag="rec", name=f"rec_{qc}")
                    nc.vector.reciprocal(rec[:], dnps[:])
                    rec_r = vec.tile([1, 512], F32R, tag="rec_r", name=f"recr_{qc}")
                    nc.vector.tensor_copy(rec_r[:], rec[:])
                    bc = ps_x.tile([128, 512], F32, tag="x", name=f"bc_{qc}")
                    nc.tensor.matmul(bc[:], ones_row_r[:], rec_r[:], start=True, stop=True)
                    bc_sb = attnp.tile([128, 512], F32, tag="bc_sb", name=f"bcsb_{qc}")
                    nc.vector.tensor_copy(bc_sb[:], bc[:])

                    # evacuate raw A^T V immediately (plain rounding copies):
                    # normalization is applied after the output projection
                    at_sb = []
                    for ct in range(CT):
                        at = attnp.tile([128, 512], MM_DT, tag=f"at_{ct}", name=f"at_{qc}_{ct}")
                        nc.vector.tensor_copy(at[:], av[ct][:])
                        at_sb.append(at)
                    # output projection; then fin = po * (1/dn) + b_out' + x
                    for ot in range(CT):
                        po = ps_x.tile([128, 512], F32, tag="x", name=f"po_{qc}_{ot}")
                        for ct in range(CT):
                            nc.tensor.matmul(
                                po[:],
                                wout_r[ct][:, 128 * ot:128 * (ot + 1)],
                                at_sb[ct][:],
                                start=(ct == 0), stop=(ct == CT - 1),
                            )
                        finf = outp.tile([128, 512], F32, tag="finf", name=f"finf_{qc}_{ot}")
                        nc.vector.tensor_tensor(out=finf[:], in0=po[:], in1=bc_sb[:], op=ALU.mult)
                        nc.vector.tensor_scalar_add(finf[:], finf[:], bo2[ot][:])
                        fin = outp.tile([128, 512], mybir.dt.bfloat16, tag="fin", name=f"fin_{qc}_{ot}")
                        nc.vector.tensor_tensor(out=fin[:], in0=finf[:], in1=xq[ot][:, qs], op=ALU.add)
                        # output writes go out on the gpsimd queue (free
                        # after the collective), keeping sync/scalar clear
                        # for the next body's input loads
                        nc.gpsimd.dma_start(out_d[128 * ot:128 * (ot + 1), qs], fin[:])

    nc.finalize()
    return nc


_NC_CACHE = None


def _get_nc(n_reps: int = 1):
    global _NC_CACHE
    if _NC_CACHE is None:
        _NC_CACHE = _build(n_reps)
    return _NC_CACHE


def kernel(x, W_qkv, b_qkv, W_out, b_out, gamma, beta):
    x = np.asarray(x, dtype=np.float32)
    W_qkv = np.asarray(W_qkv, dtype=np.float32)
    b_qkv = np.asarray(b_qkv, dtype=np.float32)
    W_out = np.asarray(W_out, dtype=np.float32)
    b_out = np.asarray(b_out, dtype=np.float32)
    gamma = np.asarray(gamma, dtype=np.float32)
    beta = np.asarray(beta, dtype=np.float32)

    nc = _get_nc()

    w_qkv_t = np.ascontiguousarray(W_qkv.T)          # [256, 768]
    w_out_t = np.ascontiguousarray(W_out.T)          # [256, 256]
    bq2 = b_qkv.reshape(3 * C, 1)
    bo2 = b_out.reshape(C, 1)
    ga2 = gamma.reshape(C, 1)
    be2 = beta.reshape(C, 1)

    xf = x.reshape(B, C, NPOS)
    in_maps = []
    for core in range(N_CORES):
        item, half = divmod(core, 2)
        xi = xf[item]
        if half == 0:
            xr = xi
        else:
            xr = np.concatenate([xi[:, NQ:], xi[:, :NQ]], axis=1)
        in_maps.append({
            "x_full": np.ascontiguousarray(xr),
            "w_qkv_t": w_qkv_t,
            "w_out_t": w_out_t,
            "b_qkv": bq2,
            "b_out": bo2,
            "gamma": ga2,
            "beta": be2,
        })

    res = bass_utils.run_bass_kernel_spmd(nc, in_maps, core_ids=list(range(N_CORES)))

    out = np.empty((B, C, NPOS), dtype=np.float32)
    for core in range(N_CORES):
        item, half = divmod(core, 2)
        out[item][:, NQ * half:NQ * (half + 1)] = np.asarray(
            res.results[core]["out"], dtype=np.float32)
    return out.reshape(B, C, H, W)
